# revision 19
# baseline (speedup 1.0000x reference)
"""CTC loss (Keras ctc_batch_cost semantics) on 8 Trainium2 NeuronCores.

Strategy:
  - Data parallel: 16 examples per core.
  - Each core runs 32 DP "chains" of 128 steps: rows 0-15 = forward lattice
    passes over t=0..127, rows 16-31 = backward (suffix) passes over
    t=in_len-1 down to in_len-128, stored state-reversed so both directions
    share one instruction stream.  Forward/backward meet at t=127/128 and the
    host combines  sum_s alpha_127(s) * beta(s)  plus the per-chain
    normalization logs into the final loss.
  - Class compaction: each example only ever reads 65 of the 1000 classes
    (its 64 labels + blank), so the host pre-gathers y_pred down to compact
    [cc=128, chain*k] columns -- already transposed (class dim on
    partitions) and already in per-chain step order (bwd chains reversed).
    On device: one contiguous DMA loads them, one one-hot matmul per chain
    expands compact classes -> 132 lattice columns, and one SBUF->SBUF DMA
    per chain redistributes [t, s] -> [chain, (t, s)].
  - Probability-domain DP with renormalization every 8 steps (scales logged,
    exact bookkeeping on host).
"""

import sys

sys.path.insert(0, "/opt/trn_rl_repo")
sys.path.insert(0, "/opt/trn_rl_repo/concourse")

import numpy as np
import ml_dtypes

import concourse.bacc as bacc
import concourse.mybir as mybir
import concourse.tile as tile
from concourse.bass_utils import run_bass_kernel_spmd

BF16 = mybir.dt.bfloat16
F32 = mybir.dt.float32
I16 = mybir.dt.int16
AOT = mybir.AluOpType
AX = mybir.AxisListType

B, T, C, L = 128, 256, 1000, 64
NCORES = 8
EXPC = B // NCORES          # examples per core (16)
NCH = 2 * EXPC              # chains per core (32): fwd + bwd
S = 2 * L + 1               # 129 lattice states
W = 132                     # padded state width
WG = W + 2                  # with 2 guard columns
W2 = 2 * W                  # E|F pair width (F = E * skip-mask shifted)
K = T // 2                  # 128 DP steps per chain
CC = 128                    # compact class width (64 labels + blank + pad)
EPS = 1e-7
NEV = K // 8                # 16 norm events
BOOST = 19                  # per-step 2**BOOST folded into emit (range centering)

_prog_cache = {}


def build_program():
    if "nc" in _prog_cache:
        return _prog_cache["nc"]
    nc = bacc.Bacc("TRN2", target_bir_lowering=False, debug=False,
                   num_devices=NCORES)
    rows = nc.dram_tensor("rows", [CC, NCH * K], BF16, kind="ExternalInput")
    gh = nc.dram_tensor("gh", [128, NCH * W2], BF16, kind="ExternalInput")
    x0d = nc.dram_tensor("x0", [NCH, WG], BF16, kind="ExternalInput")
    gfin = nc.dram_tensor("gfin", [NCH, WG], F32, kind="ExternalOutput")
    zh = nc.dram_tensor("zh", [NCH, (K + 1) * WG], BF16, kind="ExternalOutput")
    cb = nc.dram_tensor("cb", [NCH, NEV], F32, kind="ExternalOutput")

    with tile.TileContext(nc) as tc:
        with (
            tc.tile_pool(name="ps", bufs=2, space="PSUM") as psp,
            tc.tile_pool(name="fix", bufs=1) as fix,
        ):
            Z = fix.tile([NCH, (K + 1) * WG], BF16, tag="Z")
            # zero guard columns of every slot, then load slot 0 (init state)
            nc.vector.memset(Z[:].rearrange("p (k g) -> p k g", g=WG)[:, :, 0:2], 0.0)
            nc.sync.dma_start(Z[:, 0:WG], x0d[:])
            stg = fix.tile([128, NCH * W2], BF16, tag="stg")
            # E2[ch, k, 0:W] = emission E_k; E2[ch, k, W:2W] = F_k = E*skp
            E2 = fix.tile([NCH, K * W2], BF16, tag="E2")
            ghA = fix.tile([128, NCH * W2], BF16, tag="ghA")
            rtA = fix.tile([128, NCH * K], BF16, tag="rtA")
            # split the big loads so downstream work starts on chunk 0
            for q4 in range(8):
                cw = NCH * K // 8
                nc.sync.dma_start(rtA[:, q4 * cw:(q4 + 1) * cw],
                                  rows[:, q4 * cw:(q4 + 1) * cw])
                gw = NCH * W2 // 8
                nc.scalar.dma_start(ghA[:, q4 * gw:(q4 + 1) * gw],
                                    gh[:, q4 * gw:(q4 + 1) * gw])

            # ---- compact rows -> (one-hot matmul, E|F pair) -> stage ----
            dmae = [nc.sync, nc.scalar, nc.gpsimd]
            for ch in range(NCH):
                pt = psp.tile([128, W2], F32, tag="pt")
                nc.tensor.matmul(
                    pt[:],
                    rtA[:, ch * K:(ch + 1) * K],
                    ghA[:, ch * W2:(ch + 1) * W2],
                    start=True,
                    stop=True,
                )
                nc.scalar.activation(
                    stg[:, ch * W2:(ch + 1) * W2], pt[:],
                    mybir.ActivationFunctionType.Copy,
                )
                # redistribute [t, s] -> [ch, (t, s)] for this chain,
                # spread across 4 trigger engines
                dmae[ch % 3].dma_start(
                    E2[ch:ch + 1, :].rearrange("p (k s) -> p k s", s=W2),
                    stg[:, ch * W2:(ch + 1) * W2],
                )

            # ---- DP: 128 steps over all 32 chains ----
            # Z'[s] = G[s] + G[s-1] + H[s-2];  G = Z*E*rr, H = Z*F
            # renorm every 8 steps: sum accumulated inside the Z'-add (STT
            # accum_out), reciprocal folded into the next step's G multiply.
            Gb = fix.tile([NCH, WG], BF16, tag="Gb")
            nc.vector.memset(Gb[:, 0:2], 0.0)
            Hb = fix.tile([NCH, WG], BF16, tag="Hb")
            nc.vector.memset(Hb[:, 0:2], 0.0)
            U = fix.tile([NCH, W], BF16, tag="U")
            cbuf = fix.tile([NCH, NEV], F32, tag="cbuf")
            rr = fix.tile([NCH, 1], F32, tag="rr")
            gfo = fix.tile([NCH, WG], F32, tag="gfo")

            for k in range(K):
                xo = k * WG
                no = (k + 1) * WG
                eo = k * W2
                # H on GpSimd, from step inputs (hidden under DVE ops)
                nc.gpsimd.tensor_tensor(
                    Hb[:, 2:WG], Z[:, xo + 2:xo + WG],
                    E2[:, eo + W:eo + W2], AOT.mult)
                if k % 8 == 0 and k > 0:
                    # consume renorm scale from the event at step k-1
                    nc.vector.scalar_tensor_tensor(
                        Gb[:, 2:WG], Z[:, xo + 2:xo + WG], rr[:],
                        E2[:, eo:eo + W], AOT.mult, AOT.mult)
                else:
                    nc.vector.tensor_tensor(
                        Gb[:, 2:WG], Z[:, xo + 2:xo + WG],
                        E2[:, eo:eo + W], AOT.mult)
                nc.vector.tensor_tensor(
                    U[:], Gb[:, 2:WG], Gb[:, 1:WG - 1], AOT.add)
                if k % 8 == 7:
                    ev = k // 8
                    nc.vector.scalar_tensor_tensor(
                        Z[:, no + 2:no + WG], U[:], 1.0, Hb[:, 0:W],
                        AOT.mult, AOT.add, accum_out=cbuf[:, ev:ev + 1])
                    if k < K - 1:
                        nc.vector.reciprocal(rr[:], cbuf[:, ev:ev + 1])
                elif k % 8 == 0 and k > 0:
                    # H missed the rr factor; apply it here
                    nc.vector.scalar_tensor_tensor(
                        Z[:, no + 2:no + WG], Hb[:, 0:W], rr[:], U[:],
                        AOT.mult, AOT.add)
                else:
                    nc.vector.tensor_tensor(
                        Z[:, no + 2:no + WG], U[:], Hb[:, 0:W], AOT.add)
                if k == K - 1:
                    nc.vector.tensor_copy(gfo[:], Gb[:])

            nc.sync.dma_start(gfin[:], gfo[:])
            nc.sync.dma_start(zh[:], Z[:])
            nc.sync.dma_start(cb[:], cbuf[:])

    nc.compile()
    _prog_cache["nc"] = nc
    return nc


def _host_prep(y_true, y_pred, logit_len, label_len):
    # compact per-example class columns: j<64 -> labels[j], j=64 -> blank
    yp = y_pred.astype(np.float32)
    scale = np.float32(2.0 ** BOOST)
    in_maps = []
    meta = []
    for c in range(NCORES):
        e0 = c * EXPC
        # pre-gathered + pre-transposed rows [CC, NCH*K]:
        # column ch*K + k = compact prob vector of chain ch at DP step k
        rowsT = np.zeros((CC, NCH * K), ml_dtypes.bfloat16)
        for e in range(EXPC):
            b = e0 + e
            ilen = int(logit_len[b, 0])
            cols = np.concatenate([y_true[b].astype(np.int64), [C - 1]])
            sub = (yp[b][:, cols] + EPS) * scale          # [T, L+1]
            fwd = sub[:K]                                  # t = 0..127
            bwd = sub[ilen - 1 - np.arange(K)]             # t = ilen-1-k
            rowsT[:L + 1, e * K:(e + 1) * K] = \
                fwd.T.astype(ml_dtypes.bfloat16)
            rowsT[:L + 1, (EXPC + e) * K:(EXPC + e + 1) * K] = \
                bwd.T.astype(ml_dtypes.bfloat16)

        gh = np.zeros((NCH, CC, W2), np.float32)
        sk = np.zeros((NCH, W), ml_dtypes.bfloat16)
        x0 = np.zeros((NCH, WG), ml_dtypes.bfloat16)
        x0[:, 2] = 1.0
        x0[:, 3] = 1.0
        core_meta = []
        for e in range(EXPC):
            b = e0 + e
            lab = int(label_len[b, 0])
            ilen = int(logit_len[b, 0])
            labels = y_true[b].astype(np.int64)
            s_idx = np.arange(S)
            ext = np.where(s_idx % 2 == 0, C - 1,
                           labels[np.minimum(s_idx // 2, L - 1)])
            ext_m2 = np.concatenate([np.full(2, -1, np.int64), ext[:-2]])
            allow = (s_idx >= 2) & (ext != C - 1) & (ext != ext_m2)
            Sb = 2 * lab + 1

            # forward chain e: states s (one-hot cols)
            # compact class of state s: even -> 64 (blank), odd s=2m+1 -> m
            for s in range(Sb):
                cc = L if s % 2 == 0 else (s - 1) // 2
                gh[e, cc, s] = 1.0
            sk[e, :Sb] = allow[:Sb].astype(np.float32)

            # backward chain 16+e: reversed states
            r = EXPC + e
            for k2 in range(Sb):
                sorig = 2 * lab - k2
                cc = L if sorig % 2 == 0 else (sorig - 1) // 2
                gh[r, cc, k2] = 1.0
            k2v = np.arange(2, Sb)
            skr = np.zeros(W, np.float32)
            skr[k2v] = allow[2 * lab - k2v + 2]
            sk[r] = skr.astype(np.float32)
            core_meta.append((lab, ilen))

        # F one-hot = E one-hot * skip-mask shifted by 2:
        # gh[ch, cc, W + s] = gh[ch, cc, s] * sk[ch, s+2]
        skp = np.zeros((NCH, W), np.float32)
        skp[:, :W - 2] = sk.astype(np.float32)[:, 2:]
        gh[:, :, W:W2] = gh[:, :, 0:W] * skp[:, None, :]

        in_maps.append({
            "rows": rowsT,
            "gh": np.ascontiguousarray(
                gh.astype(ml_dtypes.bfloat16).transpose(1, 0, 2)
            ).reshape(128, NCH * W2),
            "x0": x0,
        })
        meta.append(core_meta)
    return in_maps, meta


def _host_finish(results, meta):
    loss = np.zeros((B, 1), np.float32)
    for c in range(NCORES):
        gf = results[c]["gfin"].astype(np.float32)
        zhr = results[c]["zh"].astype(np.float32).reshape(NCH, K + 1, WG)
        cbv = results[c]["cb"].astype(np.float64)
        for e in range(EXPC):
            lab, ilen = meta[c][e]
            Sb = 2 * lab + 1
            alpha = gf[e, 2:2 + Sb].astype(np.float64)
            q = ilen - K
            beta = zhr[EXPC + e, q, 2:2 + Sb].astype(np.float64)[::-1]
            end = float(np.dot(alpha, beta))
            # fwd G_127 consumed events 0..14 (at steps 8,16,...,120).
            # bwd slot q carries the scales consumed at steps <= q-1,
            # i.e. events 0..(q-1)//8 - 1.
            lf = np.sum(np.log(cbv[e, :15]))
            nb = (q - 1) // 8 if q >= 1 else 0
            lb = np.sum(np.log(cbv[EXPC + e, :nb])) if nb > 0 else 0.0
            boost = BOOST * np.log(2.0) * (K + q)
            loss[c * EXPC + e, 0] = -(np.log(end) + lf + lb - boost)
    return loss


def kernel(y_true, y_pred, logit_len, label_len):
    nc = build_program()
    in_maps, meta = _host_prep(y_true, y_pred, logit_len, label_len)
    res = run_bass_kernel_spmd(nc, in_maps, core_ids=list(range(NCORES)))
    return _host_finish(res.results, meta)


if __name__ == "__main__":
    import reference
    inputs = reference.setup_inputs()
    inputs = {k: np.asarray(v) for k, v in inputs.items()}
    out = kernel(**inputs)
    exp = np.asarray(reference.reference(**{k: v for k, v in inputs.items()}))
    err = np.abs(out - exp) / np.maximum(np.abs(exp), 1e-6)
    print("max rel err:", err.max(), "mean:", err.mean())
    bad = np.argsort(-err[:, 0])[:5]
    for b in bad:
        print(b, out[b, 0], exp[b, 0])


# revision 23
# speedup vs baseline: 1.3409x; 1.3409x over previous
"""CTC loss (Keras ctc_batch_cost semantics) on 8 Trainium2 NeuronCores.

Strategy:
  - Data parallel: 16 examples per core.
  - Each core runs 32 DP "chains" of 128 steps: rows 0-15 = forward lattice
    passes over t=0..127, rows 16-31 = backward (suffix) passes over
    t=in_len-1 down to in_len-128, stored state-reversed so both directions
    share one instruction stream.  Forward/backward meet at t=127/128 and the
    host combines  sum_s alpha_127(s) * beta(s)  plus the per-chain
    normalization logs into the final loss.
  - Class compaction: each example only ever reads 65 of the 1000 classes
    (its 64 labels + blank), so the host pre-gathers y_pred down to compact
    [cc=128, chain*k] columns -- already transposed (class dim on
    partitions) and already in per-chain step order (bwd chains reversed).
    On device: one contiguous DMA loads them, one one-hot matmul per chain
    expands compact classes -> 132 lattice columns, and one SBUF->SBUF DMA
    per chain redistributes [t, s] -> [chain, (t, s)].
  - Probability-domain DP with renormalization every 8 steps (scales logged,
    exact bookkeeping on host).
"""

import sys

sys.path.insert(0, "/opt/trn_rl_repo")
sys.path.insert(0, "/opt/trn_rl_repo/concourse")

import numpy as np
import ml_dtypes

import concourse.bacc as bacc
import concourse.mybir as mybir
import concourse.tile as tile
from concourse.bass_utils import run_bass_kernel_spmd

BF16 = mybir.dt.bfloat16
F32 = mybir.dt.float32
I16 = mybir.dt.int16
AOT = mybir.AluOpType
AX = mybir.AxisListType

B, T, C, L = 128, 256, 1000, 64
NCORES = 8
EXPC = B // NCORES          # examples per core (16)
NCH = 2 * EXPC              # chains per core (32): fwd + bwd
S = 2 * L + 1               # 129 lattice states
W = 132                     # padded state width
WG = W + 2                  # with 2 guard columns
K = T // 2                  # 128 DP steps per chain
CC = 128                    # compact class width (64 labels + blank + pad)
EPS = 1e-7
NEV = K // 8                # 16 norm events
BOOST = 19                  # per-step 2**BOOST folded into emit (range centering)

_prog_cache = {}


def build_program():
    if "nc" in _prog_cache:
        return _prog_cache["nc"]
    nc = bacc.Bacc("TRN2", target_bir_lowering=False, debug=False,
                   num_devices=NCORES)
    rows = nc.dram_tensor("rows", [CC, NCH * K], BF16, kind="ExternalInput")
    gh = nc.dram_tensor("gh", [128, NCH * W], BF16, kind="ExternalInput")
    skd = nc.dram_tensor("sk", [NCH, W], BF16, kind="ExternalInput")
    x0d = nc.dram_tensor("x0", [NCH, WG], BF16, kind="ExternalInput")
    gfin = nc.dram_tensor("gfin", [NCH, WG], F32, kind="ExternalOutput")
    zh = nc.dram_tensor("zh", [NCH, (K + 1) * WG], BF16, kind="ExternalOutput")
    cb = nc.dram_tensor("cb", [NCH, NEV], F32, kind="ExternalOutput")

    with tile.TileContext(nc) as tc:
        with (
            tc.tile_pool(name="ps", bufs=2, space="PSUM") as psp,
            tc.tile_pool(name="fix", bufs=1) as fix,
        ):
            SKt = fix.tile([NCH, W], BF16, tag="SKt")
            nc.sync.dma_start(SKt[:], skd[:])
            Z = fix.tile([NCH, (K + 1) * WG], BF16, tag="Z")
            # zero guard columns of every slot, then load slot 0 (init state)
            nc.vector.memset(Z[:].rearrange("p (k g) -> p k g", g=WG)[:, :, 0:2], 0.0)
            nc.sync.dma_start(Z[:, 0:WG], x0d[:])
            stg = fix.tile([128, NCH * W], BF16, tag="stg")
            E = fix.tile([NCH, K * W], BF16, tag="E")
            ghA = fix.tile([128, NCH * W], BF16, tag="ghA")
            rtA = fix.tile([128, NCH * K], BF16, tag="rtA")
            # split the big loads so downstream work starts on chunk 0;
            # rows via SP queues, one-hots via GpSimd queues (parallel)
            for q4 in range(8):
                cw = NCH * K // 8
                nc.sync.dma_start(rtA[:, q4 * cw:(q4 + 1) * cw],
                                  rows[:, q4 * cw:(q4 + 1) * cw])
                gw = NCH * W // 8
                nc.gpsimd.dma_start(ghA[:, q4 * gw:(q4 + 1) * gw],
                                    gh[:, q4 * gw:(q4 + 1) * gw])

            # ---- compact rows -> (one-hot matmul) -> stage ----
            # PSUM->SBUF copies alternate Scalar/Vector; the per-chain
            # redistribute DMA [128p(t), 132] -> [1p(ch), 128*132]
            # alternates SP/GpSimd trigger queues.
            for ch in range(NCH):
                pt = psp.tile([128, W], F32, tag="pt")
                nc.tensor.matmul(
                    pt[:],
                    rtA[:, ch * K:(ch + 1) * K],
                    ghA[:, ch * W:(ch + 1) * W],
                    start=True,
                    stop=True,
                )
                if ch % 2 == 0:
                    nc.scalar.activation(
                        stg[:, ch * W:(ch + 1) * W], pt[:],
                        mybir.ActivationFunctionType.Copy,
                    )
                else:
                    nc.vector.tensor_copy(
                        stg[:, ch * W:(ch + 1) * W], pt[:])
                (nc.sync if ch % 2 == 0 else nc.gpsimd).dma_start(
                    E[ch:ch + 1, :].rearrange("p (k s) -> p k s", s=W),
                    stg[:, ch * W:(ch + 1) * W],
                )

            # ---- DP: 128 steps over all 32 chains ----
            Gb = fix.tile([NCH, WG], BF16, tag="Gb")
            nc.vector.memset(Gb[:, 0:2], 0.0)
            U = fix.tile([NCH, W], BF16, tag="U")
            Wt = fix.tile([NCH, W], BF16, tag="Wt")
            cbuf = fix.tile([NCH, NEV], F32, tag="cbuf")
            rr = fix.tile([NCH, 1], F32, tag="rr")
            gfo = fix.tile([NCH, WG], F32, tag="gfo")

            for k in range(K):
                xo = k * WG
                no = (k + 1) * WG
                if k % 8 == 0 and k > 0:
                    # consume the renorm scale from the event at step k-1
                    nc.vector.scalar_tensor_tensor(
                        Gb[:, 2:WG], Z[:, xo + 2:xo + WG], rr[:],
                        E[:, k * W:(k + 1) * W], AOT.mult, AOT.mult)
                else:
                    nc.vector.tensor_tensor(
                        Gb[:, 2:WG], Z[:, xo + 2:xo + WG],
                        E[:, k * W:(k + 1) * W], AOT.mult)
                nc.vector.tensor_tensor(
                    U[:], Gb[:, 2:WG], Gb[:, 1:WG - 1], AOT.add)
                nc.vector.tensor_tensor(
                    Wt[:], Gb[:, 0:W], SKt[:], AOT.mult)
                if k % 8 == 7:
                    # renorm event: sum rides the Z'-add for free (STT accum)
                    ev = k // 8
                    nc.vector.scalar_tensor_tensor(
                        Z[:, no + 2:no + WG], U[:], 1.0, Wt[:],
                        AOT.mult, AOT.add, accum_out=cbuf[:, ev:ev + 1])
                    if k < K - 1:
                        nc.vector.reciprocal(rr[:], cbuf[:, ev:ev + 1])
                else:
                    nc.vector.tensor_tensor(
                        Z[:, no + 2:no + WG], U[:], Wt[:], AOT.add)
                if k == 64:
                    # slots 0..64 are final; overlap their export with the DP
                    nc.gpsimd.dma_start(zh[:, 0:65 * WG], Z[:, 0:65 * WG])
                if k == K - 1:
                    nc.vector.tensor_copy(gfo[:], Gb[:])

            nc.sync.dma_start(gfin[:], gfo[:])
            nc.sync.dma_start(zh[:, 65 * WG:], Z[:, 65 * WG:])
            nc.sync.dma_start(cb[:], cbuf[:])

    nc.compile()
    _prog_cache["nc"] = nc
    return nc


def _host_prep(y_true, y_pred, logit_len, label_len):
    # compact per-example class columns: j<64 -> labels[j], j=64 -> blank
    yp = y_pred.astype(np.float32)
    scale = np.float32(2.0 ** BOOST)
    in_maps = []
    meta = []
    for c in range(NCORES):
        e0 = c * EXPC
        # pre-gathered + pre-transposed rows [CC, NCH*K]:
        # column ch*K + k = compact prob vector of chain ch at DP step k
        rowsT = np.zeros((CC, NCH * K), ml_dtypes.bfloat16)
        for e in range(EXPC):
            b = e0 + e
            ilen = int(logit_len[b, 0])
            cols = np.concatenate([y_true[b].astype(np.int64), [C - 1]])
            sub = (yp[b][:, cols] + EPS) * scale          # [T, L+1]
            fwd = sub[:K]                                  # t = 0..127
            bwd = sub[ilen - 1 - np.arange(K)]             # t = ilen-1-k
            rowsT[:L + 1, e * K:(e + 1) * K] = \
                fwd.T.astype(ml_dtypes.bfloat16)
            rowsT[:L + 1, (EXPC + e) * K:(EXPC + e + 1) * K] = \
                bwd.T.astype(ml_dtypes.bfloat16)

        gh = np.zeros((NCH, CC, W), ml_dtypes.bfloat16)
        sk = np.zeros((NCH, W), ml_dtypes.bfloat16)
        x0 = np.zeros((NCH, WG), ml_dtypes.bfloat16)
        x0[:, 2] = 1.0
        x0[:, 3] = 1.0
        core_meta = []
        for e in range(EXPC):
            b = e0 + e
            lab = int(label_len[b, 0])
            ilen = int(logit_len[b, 0])
            labels = y_true[b].astype(np.int64)
            s_idx = np.arange(S)
            ext = np.where(s_idx % 2 == 0, C - 1,
                           labels[np.minimum(s_idx // 2, L - 1)])
            ext_m2 = np.concatenate([np.full(2, -1, np.int64), ext[:-2]])
            allow = (s_idx >= 2) & (ext != C - 1) & (ext != ext_m2)
            Sb = 2 * lab + 1

            # forward chain e: states s (one-hot cols)
            # compact class of state s: even -> 64 (blank), odd s=2m+1 -> m
            for s in range(Sb):
                cc = L if s % 2 == 0 else (s - 1) // 2
                gh[e, cc, s] = 1.0
            sk[e, :Sb] = allow[:Sb].astype(np.float32)

            # backward chain 16+e: reversed states
            r = EXPC + e
            for k2 in range(Sb):
                sorig = 2 * lab - k2
                cc = L if sorig % 2 == 0 else (sorig - 1) // 2
                gh[r, cc, k2] = 1.0
            k2v = np.arange(2, Sb)
            skr = np.zeros(W, np.float32)
            skr[k2v] = allow[2 * lab - k2v + 2]
            sk[r] = skr.astype(np.float32)
            core_meta.append((lab, ilen))

        in_maps.append({
            "rows": rowsT,
            "gh": np.ascontiguousarray(gh.transpose(1, 0, 2)).reshape(128, NCH * W),
            "sk": sk,
            "x0": x0,
        })
        meta.append(core_meta)
    return in_maps, meta


def _host_finish(results, meta):
    loss = np.zeros((B, 1), np.float32)
    for c in range(NCORES):
        gf = results[c]["gfin"].astype(np.float32)
        zhr = results[c]["zh"].astype(np.float32).reshape(NCH, K + 1, WG)
        cbv = results[c]["cb"].astype(np.float64)
        for e in range(EXPC):
            lab, ilen = meta[c][e]
            Sb = 2 * lab + 1
            alpha = gf[e, 2:2 + Sb].astype(np.float64)
            q = ilen - K
            beta = zhr[EXPC + e, q, 2:2 + Sb].astype(np.float64)[::-1]
            end = float(np.dot(alpha, beta))
            # fwd G_127 consumed events 0..14 (at steps 8,16,...,120).
            # bwd slot q carries the scales consumed at steps <= q-1,
            # i.e. events 0..(q-1)//8 - 1.
            lf = np.sum(np.log(cbv[e, :15]))
            nb = (q - 1) // 8 if q >= 1 else 0
            lb = np.sum(np.log(cbv[EXPC + e, :nb])) if nb > 0 else 0.0
            boost = BOOST * np.log(2.0) * (K + q)
            loss[c * EXPC + e, 0] = -(np.log(end) + lf + lb - boost)
    return loss


def kernel(y_true, y_pred, logit_len, label_len):
    nc = build_program()
    in_maps, meta = _host_prep(y_true, y_pred, logit_len, label_len)
    res = run_bass_kernel_spmd(nc, in_maps, core_ids=list(range(NCORES)))
    return _host_finish(res.results, meta)


if __name__ == "__main__":
    import reference
    inputs = reference.setup_inputs()
    inputs = {k: np.asarray(v) for k, v in inputs.items()}
    out = kernel(**inputs)
    exp = np.asarray(reference.reference(**{k: v for k, v in inputs.items()}))
    err = np.abs(out - exp) / np.maximum(np.abs(exp), 1e-6)
    print("max rel err:", err.max(), "mean:", err.mean())
    bad = np.argsort(-err[:, 0])[:5]
    for b in bad:
        print(b, out[b, 0], exp[b, 0])
